# revision 2
# baseline (speedup 1.0000x reference)
"""2D DCT-II (separable) kernel for Trainium2, data-parallel over 8 NeuronCores.

Problem: img [128, 1, 512, 512] f32 -> out [128, 1, 512, 512] f32 with
    out[b,0] = scale * (Cp @ img[b,0] @ Cq^T)
where Cp[p,m] = cos(pi*(2m+1)*p/1024), Cq[q,n] = cos(pi*(2n+1)*q/1024) and
scale[p,q] = (2/512)*row[p]*col[q] (1/sqrt2 on p==0 / q==0). Since M=N=512 the
two basis matrices are identical; the rank-1 scale is folded into them:
    C'[k,j] = s_k * cos(pi*(2j+1)*k/1024),  s_k = sqrt(2/512) * (1/sqrt2 if k==0 else 1)
    out[b] = C' @ img[b] @ C'^T

Per-core (16 images each): two PE matmul stages with the image/intermediate as
the stationary operand (both stages contract over the data's partition dim, so
no transposes are needed):
    stage1: Dt[n, p] = sum_m A[m, n] * C'T[m, p]   (lhsT = A tile, rhs = C'T)
    stage2: Y[p, q]  = sum_n Dt[n, p] * C'T[n, q]  (lhsT = Dt tile, rhs = C'T)
Matmuls run in float32r (TF32-like, ~11 mantissa bits) at full PE rate.
"""

import sys
import numpy as np

for _p in ("/opt/trn_rl_repo", "/opt/pypackages"):
    if _p not in sys.path:
        sys.path.append(_p)

import concourse.tile as tile  # noqa: E402
from concourse import bacc, mybir  # noqa: E402
from concourse.bass_utils import run_bass_kernel_spmd  # noqa: E402

N_CORES = 8
B_FULL = 128
S = 512  # image side
BPC = B_FULL // N_CORES  # images per core
T = S // 128  # 4 partition tiles per image side


def _basis_f32() -> np.ndarray:
    """C'T[j, k] = s_k * cos(pi*(2j+1)*k/1024), shape [512, 512] f32."""
    j = np.arange(S, dtype=np.float64)
    k = np.arange(S, dtype=np.float64)
    c = np.cos(np.pi * (2.0 * j[:, None] + 1.0) * k[None, :] / (2.0 * S))
    s = np.full(S, np.sqrt(2.0 / S), dtype=np.float64)
    s[0] /= np.sqrt(2.0)
    return (c * s[None, :]).astype(np.float32)


def _build(trace_scopes: bool = False):
    nc = bacc.Bacc("TRN2", target_bir_lowering=False, debug=False)
    img_d = nc.dram_tensor(
        "img", [BPC, S, S], mybir.dt.float32r, kind="ExternalInput"
    ).ap()
    ct_d = nc.dram_tensor("ct", [S, S], mybir.dt.float32r, kind="ExternalInput").ap()
    out_d = nc.dram_tensor("out", [BPC, S, S], mybir.dt.float32, kind="ExternalOutput").ap()

    # DRAM views tiled to 128 partitions: [img, tile, 128, S]
    img_v = img_d.rearrange("b (t p) n -> b p t n", p=128)
    out_v = out_d.rearrange("b (t p) q -> b p t q", p=128)

    with tile.TileContext(nc) as tc:
        with (
            tc.tile_pool(name="const", bufs=1) as cpool,
            tc.tile_pool(name="a", bufs=3) as apool,
            tc.tile_pool(name="dt", bufs=2) as dtpool,
            tc.tile_pool(name="o", bufs=2) as opool,
            tc.tile_pool(name="ps1", bufs=4, space="PSUM") as ps1pool,
            tc.tile_pool(name="ps2", bufs=4, space="PSUM") as ps2pool,
        ):
            # C'T in SBUF: tile mt holds rows j in [mt*128,(mt+1)*128), all k.
            ct_sb = cpool.tile([128, T, S], mybir.dt.float32r)
            nc.sync.dma_start(ct_sb[:], ct_d.rearrange("(t p) k -> p t k", p=128))

            for i in range(BPC):
                a_sb = apool.tile([128, T, S], mybir.dt.float32r, tag="a")
                nc.sync.dma_start(a_sb[:], img_v[i])

                # stage 1: Dt[n, p] = sum_m A[m, n] C'T[m, p]
                ps1 = [ps1pool.tile([128, S], mybir.dt.float32, tag="ps1", name=f"ps1_{i}_{j}") for j in range(T)]
                for nt in range(T):
                    for mt in range(T):
                        nc.tensor.matmul(
                            ps1[nt][:],
                            a_sb[:, mt, nt * 128 : (nt + 1) * 128],
                            ct_sb[:, mt, :],
                            start=(mt == 0),
                            stop=(mt == T - 1),
                        )
                dt_sb = dtpool.tile([128, T, S], mybir.dt.float32r, tag="dt")
                for nt in range(T):
                    nc.vector.tensor_copy(dt_sb[:, nt, :], ps1[nt][:])

                # stage 2: Y[p, q] = sum_n Dt[n, p] C'T[n, q]
                ps2 = [ps2pool.tile([128, S], mybir.dt.float32, tag="ps2", name=f"ps2_{i}_{j}") for j in range(T)]
                for pt in range(T):
                    for nt in range(T):
                        nc.tensor.matmul(
                            ps2[pt][:],
                            dt_sb[:, nt, pt * 128 : (pt + 1) * 128],
                            ct_sb[:, nt, :],
                            start=(nt == 0),
                            stop=(nt == T - 1),
                        )
                o_sb = opool.tile([128, T, S], mybir.dt.float32, tag="o")
                for pt in range(T):
                    nc.scalar.copy(o_sb[:, pt, :], ps2[pt][:])
                nc.sync.dma_start(out_v[i], o_sb[:])
    nc.compile()
    return nc


_NC_CACHE = None


def _get_nc():
    global _NC_CACHE
    if _NC_CACHE is None:
        _NC_CACHE = _build()
    return _NC_CACHE


def run_sharded(img: np.ndarray, **spmd_kwargs):
    """img [128, 1, 512, 512] f32 -> (out [128, 1, 512, 512] f32, BassKernelResults)."""
    img = np.ascontiguousarray(np.asarray(img, dtype=np.float32)).reshape(B_FULL, S, S)
    ct = _basis_f32()
    nc = _get_nc()
    in_maps = [
        {"img": img[k * BPC : (k + 1) * BPC], "ct": ct} for k in range(N_CORES)
    ]
    res = run_bass_kernel_spmd(nc, in_maps, core_ids=list(range(N_CORES)), **spmd_kwargs)
    out = np.empty((B_FULL, S, S), dtype=np.float32)
    for k in range(N_CORES):
        out[k * BPC : (k + 1) * BPC] = res.results[k]["out"]
    return out.reshape(B_FULL, 1, S, S), res


def kernel(img: np.ndarray) -> np.ndarray:
    out, _ = run_sharded(img)
    return out
